# revision 1
# baseline (speedup 1.0000x reference)
"""Trainium2 Bass kernel: parity-polynomial segment_reduce.

Reference math:
    spins = 1 - 2*bits                                   # {-1,+1}
    parities[b,t] = prod_o spins_pad[b, idx_pad[t,o]]    # [B, T]
    out[b] = parities[b] @ theta

Every parity factor is (-1)^{bit}, so
    out[b] = sum_t theta[t] * (-1)^{popcount(key[b] & mask[t])}
with key[b] = sum_i bits[b,i]<<i and mask[t] = XOR-fold of (1<<idx_pad[t,o])
(the pad index NUM_BITS maps to a constant-one column, i.e. contributes no bit;
a repeated index squares to +1, which XOR-folding reproduces).

For this problem idx_pad only references bits 0..11, so every mask < 4096 and
out[b] = f(key12[b]) where f = WHT_4096(theta_spread) — a 4096-point
Walsh-Hadamard transform of theta scattered by mask.  On device (per core,
batch-sharded 512 rows):

  1. WHT via the 6/6 Kronecker split (p = key>>6, c = key&63):
         F64[p,c] = (H64 @ Ts @ H64)[p,c],  Ts[q,md] = theta_spread[q*64+md].
     The second stage uses an augmented stationary (zeros | H64) so F64
     lands on PSUM partitions 64:128, matching the p-one-hot rows.
  2. BOTH sub-key grids from ONE bf16 matmul stacked on 128 partitions
     (rows j<64: c(b)-j, rows 64+i: p(b)-i; the constant-ones bits row
     carries the -j offsets; all operands bf16-exact small integers, PSUM
     accumulates fp32), and ONE is_equal turns the stack into both one-hots
     -- this halves the key-matmul and one-hot cost of the 7/5 version.
  3. One-hot via is_equal against immediate 0.0 (the pointer-scalar
     tensor_scalar form has no sync-wait slot in the ISA).
  4. Gather F rows with a one-hot matmul, mask columns with the c one-hot,
     then column-reduce with FOUR 1-column ones-matmuls whose stationaries
     are 128-column slices of the product -- the batch lands on the output
     partitions, so the final PSUM->SBUF staging copy is [128, 4] instead
     of [1, 512] (~520ns less DVE time on the critical tail):
         out[i,g] = sum_c prod[c, g*128+i],  prod = (F64^T@onehot_p)*onehot_c.

Sync-slot discipline (walrus "Too many sync wait commands"): inputs are packed
into 2 DMAs (each extra DMA also costs a serialized ~625ns HWDGE
descriptor-generation slot), every PSUM->SBUF staging copy runs on DVE only,
and a warm-up matmul lets PE observe each DMA semaphore before the consuming
matmul, so no instruction ever needs more than one new cross-engine wait.

Host does only sharding, dtype/layout staging, and the index bookkeeping
(mask XOR-fold + theta scatter).  All theta- and bit-dependent arithmetic
runs on device.
"""

import numpy as np

B, NUM_BITS, ORDER = 4096, 32, 12
N_CORES = 8
B_LOCAL = B // N_CORES          # 512
KEYS = 1 << ORDER               # 4096
PC = 64                         # 6/6 split: p = key>>6, c = key&63
PC_BITS = 6
ROWS = NUM_BITS + 1             # bits rows + constant-ones row
BB_COLS = B_LOCAL + 2 * PC      # bitsT | W (c-grid 64 | p-grid 64)
PKK_COLS = 4 * PC               # thetaT64 | H64 | h64aug(zeros|H64)

_STATE = {}


def _sylvester(n):
    """H[i,j] = (-1)^popcount(i&j), Sylvester ordering."""
    h = np.array([[1.0]], dtype=np.float32)
    while h.shape[0] < n:
        h = np.block([[h, h], [h, -h]])
    return np.ascontiguousarray(h, dtype=np.float32)


def _build_module():
    import concourse.mybir as mybir
    import concourse.tile as tile
    from concourse import bacc

    f32 = mybir.dt.float32
    bf16 = mybir.dt.bfloat16
    nc = bacc.Bacc(
        "TRN2",
        target_bir_lowering=False,
        debug=False,
        enable_asserts=True,
        num_devices=N_CORES,
    )

    bb = nc.dram_tensor("bb", [ROWS, BB_COLS], bf16, kind="ExternalInput").ap()
    # thetaT64 | H64 | h64aug merged into ONE DMA: each extra DMA costs a
    # full serialized HWDGE descriptor-generation slot (~625ns)
    pkk = nc.dram_tensor(
        "pkk", [PC, PKK_COLS], f32, kind="ExternalInput").ap()
    out = nc.dram_tensor("out", [128, 4], f32, kind="ExternalOutput").ap()

    with tile.TileContext(nc) as tc:
        with (
            tc.tile_pool(name="sb", bufs=1) as sb,
            tc.tile_pool(name="ps", bufs=1, space="PSUM") as ps,
        ):
            t_bb = sb.tile([ROWS, BB_COLS], bf16)
            nc.sync.dma_start(out=t_bb, in_=bb)
            t_pkk = sb.tile([PC, PKK_COLS], f32)
            nc.sync.dma_start(out=t_pkk, in_=pkk)

            t_bitsT = t_bb[:, 0:B_LOCAL]
            t_W = t_bb[:, B_LOCAL : B_LOCAL + 2 * PC]
            t_thetaT = t_pkk[:, 0:PC]
            t_h64 = t_pkk[:, PC : 2 * PC]
            t_h64aug = t_pkk[:, 2 * PC : 4 * PC]   # [64, 128]: zeros | H64

            t_ones = sb.tile([PC, 1], bf16)
            nc.vector.memset(t_ones, 1.0)

            # --- WHT of theta_spread: F64 = H64 @ Ts @ H64 (fp32).  The
            # second stage uses an augmented stationary (zeros | H64) so F64
            # lands on PSUM partitions 64:128, matching the p-one-hot rows of
            # the stacked grid (matmul operands must share base partition).
            p_G = ps.tile([PC, PC], f32)
            nc.tensor.matmul(p_G, t_thetaT, t_h64)          # waits: pkk DMA
            p_warm = ps.tile([1, 1], f32)
            nc.tensor.matmul(p_warm, t_pkk[:, PC : PC + 1], t_pkk[:, PC : PC + 1])
            t_G = sb.tile([PC, PC], f32)
            nc.vector.tensor_copy(t_G, p_G)
            p_F = ps.tile([2 * PC, PC], f32)
            nc.tensor.matmul(p_F, t_h64aug, t_G)            # waits: DVE only
            t_F = sb.tile([2 * PC, PC], bf16)
            nc.vector.tensor_copy(t_F, p_F)                 # fp32 -> bf16

            # --- BOTH sub-keys minus index, split into batch halves so the
            # t_G staging copy threads between the one-hot halves on DVE ---
            HB = B_LOCAL // 2
            p_k = ps.tile([2 * PC, B_LOCAL], f32)
            nc.tensor.matmul(p_k[:, 0:HB], t_W, t_bitsT[:, 0:HB])
            nc.tensor.matmul(p_k[:, HB:B_LOCAL], t_W, t_bitsT[:, HB:B_LOCAL])
            t_oh = sb.tile([2 * PC, B_LOCAL], bf16)
            nc.vector.tensor_scalar(
                out=t_oh[:, 0:HB], in0=p_k[:, 0:HB],
                scalar1=0.0, scalar2=None, op0=mybir.AluOpType.is_equal)
            nc.vector.tensor_scalar(
                out=t_oh[:, HB:B_LOCAL], in0=p_k[:, HB:B_LOCAL],
                scalar1=0.0, scalar2=None, op0=mybir.AluOpType.is_equal)

            # --- gather + reduce:  out[b] = F64[p_b, c_b] ---
            p_o1 = ps.tile([PC, B_LOCAL], f32)
            nc.tensor.matmul(
                p_o1, t_F[PC : 2 * PC, :], t_oh[PC : 2 * PC, :])
            t_prod = sb.tile([PC, B_LOCAL], bf16)
            nc.vector.tensor_mul(t_prod, p_o1, t_oh[0:PC, :])
            # column sums, batch transposed onto output partitions: four
            # 1-column matmuls (stationary = a 128-column slice of prod)
            # leave only a [128, 4] PSUM->SBUF copy instead of [1, 512]
            p_out = ps.tile([128, 4], f32)
            for g in range(4):
                nc.tensor.matmul(
                    p_out[:, g : g + 1],
                    t_prod[:, g * 128 : (g + 1) * 128], t_ones)
            t_out = sb.tile([128, 4], f32)
            nc.vector.tensor_copy(t_out, p_out)
            nc.sync.dma_start(out=out, in_=t_out)

    nc.compile()
    return nc


def _get_module():
    nc = _STATE.get("nc")
    if nc is None:
        nc = _build_module()
        _STATE["nc"] = nc
    return nc


def _host_prep(bitstrings, theta, idx_pad):
    """Index bookkeeping + input staging. Returns per-core input maps."""
    import ml_dtypes

    bitstrings = np.asarray(bitstrings)
    theta = np.asarray(theta, dtype=np.float32)
    idx_pad = np.asarray(idx_pad).astype(np.int64)

    # mask[t] = XOR-fold of one-hot bit positions (pad index >= NUM_BITS -> no bit)
    onehots = np.where(idx_pad >= NUM_BITS, 0, np.int64(1) << np.clip(idx_pad, 0, 62))
    masks = np.bitwise_xor.reduce(onehots, axis=1)
    if masks.size and int(masks.max()) >= KEYS:
        raise NotImplementedError(
            "kernel specialized for masks spanning bits 0..11 "
            f"(max mask {int(masks.max())})"
        )
    theta_spread = np.zeros(KEYS, np.float32)
    np.add.at(theta_spread, masks, theta)

    # Stationary key weights.  Column j (j<64) computes c(b)-j, column 64+j
    # computes p(b)-j; row 32 multiplies the constant-ones bit row and
    # carries the -j offsets.
    W = np.zeros((ROWS, 2 * PC), np.float32)
    for k in range(PC_BITS):
        W[k, 0:PC] = float(1 << k)
    for k in range(PC_BITS, ORDER):
        W[k, PC : 2 * PC] = float(1 << (k - PC_BITS))
    W[NUM_BITS, 0:PC] = -np.arange(PC, dtype=np.float32)
    W[NUM_BITS, PC : 2 * PC] = -np.arange(PC, dtype=np.float32)

    h64 = _sylvester(PC)
    pkk = np.zeros((PC, PKK_COLS), np.float32)
    pkk[:, 0:PC] = theta_spread.reshape(PC, PC).T      # thetaT64[md, q]
    pkk[:, PC : 2 * PC] = h64
    pkk[:, 3 * PC : 4 * PC] = h64                      # h64aug = zeros | H64

    base = {"pkk": pkk}

    bits_f = bitstrings.astype(np.float32)
    in_maps = []
    for c in range(N_CORES):
        m = dict(base)
        bbuf = np.ones((ROWS, BB_COLS), np.float32)
        bbuf[:NUM_BITS, 0:B_LOCAL] = bits_f[c * B_LOCAL : (c + 1) * B_LOCAL, :].T
        bbuf[:, B_LOCAL:] = W
        m["bb"] = bbuf.astype(ml_dtypes.bfloat16)
        in_maps.append(m)
    return in_maps


def kernel(bitstrings, theta, idx_pad):
    from concourse.bass_utils import run_bass_kernel_spmd

    in_maps = _host_prep(bitstrings, theta, idx_pad)
    nc = _get_module()
    res = run_bass_kernel_spmd(nc, in_maps, core_ids=list(range(N_CORES)))
    # out[i, g] holds sample b_local = g*128 + i
    out = np.concatenate([np.asarray(r["out"]).T.ravel() for r in res.results])
    return out.astype(np.float32)



# revision 48
# speedup vs baseline: 1.2623x; 1.2623x over previous
"""Trainium2 Bass kernel: parity-polynomial segment_reduce.

Reference math:
    spins = 1 - 2*bits                                   # {-1,+1}
    parities[b,t] = prod_o spins_pad[b, idx_pad[t,o]]    # [B, T]
    out[b] = parities[b] @ theta

Every parity factor is (-1)^{bit}, so
    out[b] = sum_t theta[t] * (-1)^{popcount(key[b] & mask[t])}
with key[b] = sum_i bits[b,i]<<i and mask[t] = XOR-fold of (1<<idx_pad[t,o]).
For this problem every mask < 4096, so with key = (p<<6)|c, mask = (q<<6)|d:

    out[b] = sum_{q,d} TsM[q,d] * H[q,p_b] * H[d,c_b]
           = sum_d v[d,b] * M1[d,b]

where TsM = theta_spread.reshape(64,64), H = Sylvester-Hadamard-64,
A = H @ TsM (the only theta-side compute: ONE 64x64 matmul),
M1[d,b] = A[p_b,d] (a one-hot gather matmul over the p-side), and
v[d,b] = (-1)^{popcount(d & c_b)} is a sign matrix computed directly from the
c-bits as vhat = (pc mod 2) - 0.5 = -v/2 (one DVE tensor_scalar; the -2 is
folded into H on the host).

The 512-sample batch is split into two groups at matmul partition bases 0/32,
and every elementwise stage is STACKED to [128, 256] (group0 on PSUM
partitions 0:64, group1 on 64:128) via zero-padded stationaries +
accumulating matmuls, so is_equal, the sign op, and the product are ONE
128-partition DVE op each (DVE cost scales with the free axis only).

Per core:
  1. ONE input DMA [64, 640] bf16 (SP/HWDGE).
  2. PE: warm-ups seed the p-state ramp; A-matmul; 2+2 accumulating key
     matmuls -> p_k (p(b)-j offsets, both groups) and p_vk (popcount counts).
  3. DVE: is_equal -> OHp stack; (mod 2) - 0.5 -> vhat stack.
     Act: copies A into the two diagonal blocks of a [128,128] stationary.
  4. PE: two accumulating gather matmuls -> M1 stack; DVE: prod = M1 * vhat;
     PE: four 1-column ones-matmuls column-reduce prod into p_out[128, 4]
     (sample g*128+i lands on partition i, column g).
  5. DVE stages p_out -> SBUF; a PREPARED kv_writeback (descriptors generated
     on Pool during the input-DMA wait) is fired by trigger_dma: the
     post-compute tail is trigger + transfer + DMA-sem, skipping the 625ns
     HWDGE gen + 650ns DGE->DMA delay of a plain dma_start.

Host does only sharding, dtype/layout staging, and index bookkeeping
(mask XOR-fold + theta scatter).  All theta- and bit-dependent arithmetic
runs on device.
"""

import numpy as np

B, NUM_BITS, ORDER = 4096, 32, 12
N_CORES = 8
B_LOCAL = B // N_CORES          # 512
KEYS = 1 << ORDER               # 4096
PC = 64                         # 6/6 split: p = key>>6, c = key&63
PC_BITS = 6
GROUPS = 2                      # batch groups row-stacked at partition 0 / 32
GROW = 32                       # group g rows start at g*GROW (matmul base rule)
GB = B_LOCAL // GROUPS          # 256 samples per group
ROWS = PC_BITS * 2 + 2          # 12 bit rows + two ones rows = 14
IN_COLS = GB + 5 * PC           # bits 256 | statP 64 | statQ 64 | statC 64 | H | Ts

_STATE = {}


def _sylvester(n):
    """H[i,j] = (-1)^popcount(i&j), Sylvester ordering."""
    h = np.array([[1.0]], dtype=np.float32)
    while h.shape[0] < n:
        h = np.block([[h, h], [h, -h]])
    return np.ascontiguousarray(h, dtype=np.float32)


def _build_module():
    import bass_rust as _bass_rust
    import concourse.mybir as mybir
    import concourse.tile as tile
    from concourse import bacc

    f32 = mybir.dt.float32
    bf16 = mybir.dt.bfloat16
    i32 = mybir.dt.int32
    nc = bacc.Bacc(
        "TRN2",
        target_bir_lowering=False,
        debug=False,
        enable_asserts=True,
        num_devices=N_CORES,
    )

    inp = nc.dram_tensor("inp", [PC, IN_COLS], bf16, kind="ExternalInput").ap()
    # kv layout [batch=1, d_head=128, d_head_outer=1, n_ctx=4] is flat
    # f32[p*4+g] == p_out[p, g], identical to a plain [128, 4] tensor.
    out = nc.dram_tensor("out", [1, 128, 1, 4], f32, kind="ExternalOutput").ap()
    BISECT_PLAIN_DMA = False

    C_SP = GB                  # stat-P block (p-offsets)
    C_SQ = GB + PC             # stat-Q block: pc/2 - 0.25 + 2^23 (RNE floor)
    C_SC = GB + 2 * PC         # stat-C block (popcount weights)
    C_H = GB + 3 * PC          # -2*H64
    C_TS = GB + 4 * PC         # TsM

    with tile.TileContext(nc) as tc:
        with (
            tc.tile_pool(name="sb", bufs=1) as sb,
            tc.tile_pool(name="ps", bufs=1, space="PSUM") as ps,
        ):
            # --- pre-DMA setup: constants + PE p-state warm-up ---------------
            t_w = sb.tile([1, 1], bf16)
            nc.vector.memset(t_w, 1.0)
            t_ones = sb.tile([128, 1], f32)
            nc.vector.memset(t_ones, 1.0)
            t_A0 = sb.tile([PC, PC], bf16)
            t_A1 = sb.tile([128, PC], bf16)
            t_out = sb.tile([128, 1, 1, 4], f32)
            t_ctx = sb.tile([128, 1], i32)
            nc.gpsimd.memset(t_ctx, 0)
            if not BISECT_PLAIN_DMA:
                dma_sem = nc.alloc_semaphore("out_dma")
                nc.gpsimd.kv_writeback(
                    out, t_out[:], t_ctx[:], prepare_only=True, sem=dma_sem)
            # warm-ups target p_out (overwritten by the column sums later);
            # PSUM has only 8 banks and each tile takes a full bank
            p_out = ps.tile([128, 4], f32)
            nc.tensor.matmul(p_out[0:1, 0:1], t_w, t_w)
            nc.tensor.matmul(p_out[0:1, 0:1], t_w, t_w)

            t_in = sb.tile([PC, IN_COLS], bf16)
            nc.sync.dma_start(out=t_in, in_=inp)

            # --- theta side: A = (-2H) @ TsM, staged into the two diagonal
            # blocks of the M1 stationary (Act; GPSIMD can't touch PSUM)
            p_A = ps.tile([PC, PC], f32)
            nc.tensor.matmul(p_A, t_in[:, C_H : C_H + PC], t_in[:, C_TS : C_TS + PC])
            # both gather stationary blocks staged by DVE in its idle window
            # before is_equal's input is ready (separate tiles: a same-tile
            # WAW would cost a blocking self-sem); Act runs only s1
            nc.vector.tensor_copy(t_A0, p_A)
            nc.vector.tensor_copy(t_A1[PC : 2 * PC, :], p_A)

            # --- bits side: key matmuls stack group0 on PSUM partitions
            # 0:64 and group1 on 64:128 (64-col stationaries; the PSUM out AP
            # carries the partition base -- HW-verified, accumulation across
            # different stationary bases is NOT)
            p_k = ps.tile([2 * PC, GB], f32)
            p_q1 = ps.tile([2 * PC, GB], f32)
            p_vk = ps.tile([2 * PC, GB], f32)
            r0 = slice(0, ROWS)
            r1 = slice(GROW, GROW + ROWS)
            for p_dst, c_base in ((p_q1, C_SQ), (p_k, C_SP), (p_vk, C_SC)):
                nc.tensor.matmul(p_dst[0:PC, :], t_in[r0, c_base : c_base + PC],
                                 t_in[r0, 0:GB])
                nc.tensor.matmul(p_dst[PC : 2 * PC, :],
                                 t_in[r1, c_base : c_base + PC],
                                 t_in[r1, 0:GB])

            t_oh = sb.tile([2 * PC, GB], bf16)
            nc.vector.tensor_scalar(
                out=t_oh, in0=p_k,
                scalar1=0.0, scalar2=None, op0=mybir.AluOpType.is_equal)
            # sign contraction operand: vhat = parity(pc) - 0.5, parity from
            # the fp32 round-to-nearest-even floor baked into the q1 matmul:
            #   q1(PSUM) = 2^23 + floor(pc/2)   (exactly; HW-verified)
            #   s1 = q1*-2 + 2^24 = -2*floor(pc/2)
            #   vhat = (pc - 0.5) + s1 = parity - 0.5  (exact, in {-.5, +.5})
            t_s1 = sb.tile([2 * PC, GB], f32)
            nc.scalar.activation(
                t_s1, p_q1, mybir.ActivationFunctionType.Copy,
                bias=float(2.0**24), scale=-2.0)
            t_vst = sb.tile([2 * PC, GB], f32)
            nc.vector.scalar_tensor_tensor(
                out=t_vst, in0=p_vk, scalar=-0.5, in1=t_s1,
                op0=mybir.AluOpType.add, op1=mybir.AluOpType.add)

            # --- gather + sign-contraction: out[b] = sum_d M1[d,b]*v[d,b] ----
            p_M1 = ps.tile([2 * PC, GB], f32)
            nc.tensor.matmul(p_M1[PC : 2 * PC, :], t_A1[PC : 2 * PC, :],
                             t_oh[PC : 2 * PC, :])
            nc.tensor.matmul(p_M1[0:PC, :], t_A0, t_oh[0:PC, :])
            # tiny spacer op: Tile inserts a blocking self-sem when an op
            # reads its IMMEDIATE DVE predecessor's output; with the spacer
            # the vhat write is two ops back and prod dispatches freely
            t_dmy = sb.tile([1, 1], f32)
            nc.vector.tensor_copy(t_dmy, t_s1[0:1, 0:1])
            t_prod = sb.tile([2 * PC, GB], f32)
            nc.vector.tensor_mul(t_prod, p_M1, t_vst)

            # column sums, batch transposed onto output partitions: four
            # 1-column matmuls (stationary = a 128-column slice of prod)
            for g in range(4):
                rows = slice((g // 2) * PC, (g // 2 + 1) * PC)
                cols = slice((g % 2) * 128, (g % 2 + 1) * 128)
                nc.tensor.matmul(p_out[:, g : g + 1],
                                 t_prod[rows, cols], t_ones[rows, :])

            # the trigger is sequencer-only and Tile's deferred-RAW machinery
            # only covers producers issued BEFORE the prep, so attach an
            # explicit sync dependency on the staging copy
            cp = nc.vector.tensor_copy(t_out[:, 0, 0, :], p_out)
            if BISECT_PLAIN_DMA:
                nc.sync.dma_start(out=out, in_=t_out)
            else:
                trig = nc.gpsimd.trigger_dma(count=None)
                deps = _bass_rust.InstructionNameOrderedSet()
                deps.add(cp.ins.name)
                trig.ins.add_sync_dependencies_from(deps)
                wt = nc.gpsimd.wait_ge(dma_sem, 16)
                tdeps = _bass_rust.InstructionNameOrderedSet()
                tdeps.add(trig.ins.name)
                wt.ins.add_nosync_dependencies_from(tdeps)

    # Tile put the kv_writeback prep on the DMASW0 lane and emitted exit
    # waits on that lane's sem, but with a user completion sem (sem=) nothing
    # ever bumps it -> deadlock.  Pool's explicit wait_ge(out_dma)>=16 plus
    # the final all-engine barrier already guarantee the transfer landed
    # before teardown, so drop the orphaned lane waits.
    for blk in nc.m.functions[0].blocks:
        for inst in blk.instructions:
            si = inst.sync_info
            if si is None or not si.on_wait:
                continue
            if any(w.ant_name and w.ant_name.startswith("DMASW")
                   for w in si.on_wait):
                si.on_wait = [
                    w for w in si.on_wait
                    if not (w.ant_name and w.ant_name.startswith("DMASW"))
                ]

    nc.compile()
    return nc


def _get_module():
    nc = _STATE.get("nc")
    if nc is None:
        nc = _build_module()
        _STATE["nc"] = nc
    return nc


def _host_prep(bitstrings, theta, idx_pad):
    """Index bookkeeping + input staging. Returns per-core input maps."""
    import ml_dtypes

    bitstrings = np.asarray(bitstrings)
    theta = np.asarray(theta, dtype=np.float32)
    idx_pad = np.asarray(idx_pad).astype(np.int64)

    # mask[t] = XOR-fold of one-hot bit positions (pad index >= NUM_BITS -> no bit)
    onehots = np.where(idx_pad >= NUM_BITS, 0, np.int64(1) << np.clip(idx_pad, 0, 62))
    masks = np.bitwise_xor.reduce(onehots, axis=1)
    if masks.size and int(masks.max()) >= KEYS:
        raise NotImplementedError(
            "kernel specialized for masks spanning bits 0..11 "
            f"(max mask {int(masks.max())})"
        )
    theta_spread = np.zeros(KEYS, np.float32)
    np.add.at(theta_spread, masks, theta)
    ts_m = theta_spread.reshape(PC, PC)                 # TsM[q, d]

    # Key-matmul stationaries [14, 64] each (rows 12/13 are ones-rows):
    #   p-off[:, j] : p(b)-j  = sum_{k=6..11} 2^(k-6) bit_k  +  (-j)*1
    #   cnt[:, d]   : pc = popcount(d & c_b) = sum_{k=0..5} dbit_k bit_k
    #   q1[:, d]    : pc/2 - 0.25 + 2^23 -> fp32 PSUM rounds (RNE) to
    #                 2^23 + floor(pc/2); constants split across the two
    #                 ones-rows (each bf16-exact), 2^23 last in partition
    #                 order so the single rounding happens at the end
    w_p = np.zeros((ROWS, PC), np.float32)
    for k in range(PC_BITS):
        w_p[PC_BITS + k, :] = float(1 << k)
    w_p[2 * PC_BITS, :] = -np.arange(PC, dtype=np.float32)
    d_idx = np.arange(PC)
    w_c = np.zeros((ROWS, PC), np.float32)
    for k in range(PC_BITS):
        w_c[k, :] = ((d_idx >> k) & 1).astype(np.float32)
    w_q = 0.5 * w_c
    w_q[2 * PC_BITS, :] = -0.25
    w_q[2 * PC_BITS + 1, :] = float(2.0**23)

    h64 = _sylvester(PC)

    C_SP = GB
    C_SQ = GB + PC
    C_SC = GB + 2 * PC
    C_H = GB + 3 * PC
    C_TS = GB + 4 * PC

    base = np.zeros((PC, IN_COLS), np.float32)
    base[:, C_H : C_H + PC] = -2.0 * h64
    base[:, C_TS : C_TS + PC] = ts_m
    # group g's stationaries live at rows g*GROW (sharing the moving base)
    for g in range(GROUPS):
        rows = slice(g * GROW, g * GROW + ROWS)
        base[rows, C_SP : C_SP + PC] = w_p
        base[rows, C_SQ : C_SQ + PC] = w_q
        base[rows, C_SC : C_SC + PC] = w_c

    bits_f = bitstrings[:, :ORDER].astype(np.float32)
    in_maps = []
    for c in range(N_CORES):
        buf = base.copy()
        for g in range(GROUPS):
            rows = slice(g * GROW, g * GROW + ORDER)
            s0 = c * B_LOCAL + g * GB
            buf[rows, 0:GB] = bits_f[s0 : s0 + GB, :].T
            buf[g * GROW + ORDER, 0:GB] = 1.0
            buf[g * GROW + ORDER + 1, 0:GB] = 1.0
        in_maps.append({"inp": buf.astype(ml_dtypes.bfloat16)})
    return in_maps


def kernel(bitstrings, theta, idx_pad):
    from concourse.bass_utils import run_bass_kernel_spmd

    in_maps = _host_prep(bitstrings, theta, idx_pad)
    nc = _get_module()
    res = run_bass_kernel_spmd(nc, in_maps, core_ids=list(range(N_CORES)))
    # out flat f32[i*4 + g] holds sample b_local = g*128 + i
    out = np.concatenate(
        [np.asarray(r["out"]).reshape(128, 4).T.ravel() for r in res.results])
    return out.astype(np.float32)
